# revision 1
# baseline (speedup 1.0000x reference)
"""LocallyConnected2d (B=8, C_in=32, 48x48, C_out=32, 3x3, pad 1) on 8 trn2 cores.

Strategy: shard the spatial-location axis L = H*W across cores (6 image rows
each). Per location l the op is an (8x288)@(288x32) GEMM with location-unique
weights; weight streaming (85 MB total) dominates -> memory-bound.

Device mapping per core:
  - x halo slice lives in SBUF replicated 3x with kw column shifts, laid out
    [p=(kw*32+c), (row, col, b)], so the im2col patch for any location is a
    plain strided AP slice (no patch materialization).
  - Contraction (d=288) is split into 3 kh-rounds of K=96=(3 kw x 32 c),
    PSUM-accumulated. Round 0 carries a 97th row: ones in x, transposed bias
    in W, folding the bias add into the matmul.
  - Per location: stationary = x-view [K,8(b)] (8-column LDW, cheap),
    moving = W slice [K,32(o)], out = PSUM [8(b),32(o)].
  - Output assembled in SBUF as [b, (o, r, q)] and stored with one DMA.
"""

import numpy as np

import concourse.bacc as bacc
import concourse.tile as tile
from concourse import mybir
from concourse.bass_utils import run_bass_kernel_spmd

B, C_IN, H, W = 8, 32, 48, 48
C_OUT = 32
N_CORES = 8
RP = H // N_CORES  # rows per core (6)
LP = RP * W  # locations per core (288)
F32 = mybir.dt.float32

_nc = None


def _build():
    nc = bacc.Bacc(
        "TRN2", target_bir_lowering=False, debug=False, num_devices=N_CORES
    )
    xh = nc.dram_tensor("xh", [C_IN, RP + 2, W + 2, B], F32, kind="ExternalInput")
    w = nc.dram_tensor("w", [C_IN * 9, LP, C_OUT], F32, kind="ExternalInput")
    bt = nc.dram_tensor("bt", [LP, C_OUT], F32, kind="ExternalInput")
    out = nc.dram_tensor("out", [B, C_OUT, RP, W], F32, kind="ExternalOutput")

    # w rows are d = c*9 + kh*3 + kw; expose (kh, kw, c) so one DMA per
    # (kh, image-row) lands as SBUF partitions p = kw*32 + c.
    wr = w.rearrange("(c kh kw) l o -> kh kw c l o", c=C_IN, kh=3, kw=3)

    with tile.TileContext(nc) as tc:
        with (
            tc.tile_pool(name="xpool", bufs=1) as xpool,
            tc.tile_pool(name="wpool", bufs=6) as wpool,
            tc.tile_pool(name="opool", bufs=1) as opool,
            tc.tile_pool(name="pspool", bufs=6, space="PSUM") as pspool,
        ):
            x3 = xpool.tile([128, (RP + 2) * W * B], F32)
            for kw in range(3):
                nc.sync.dma_start(
                    x3[kw * 32 : (kw + 1) * 32, :], xh[:, :, kw : kw + W, :]
                )
            nc.vector.memset(x3[96:97, :], 1.0)

            out_sb = opool.tile([B, C_OUT * RP * W], F32)
            out_v = out_sb[:, :].rearrange(
                "p (o r q) -> p r q o", o=C_OUT, r=RP, q=W
            )

            for rl in range(RP):
                wts = []
                for kh in range(3):
                    wt = wpool.tile([128, W * C_OUT], F32, tag="wt")
                    nc.sync.dma_start(
                        wt[0:96, :], wr[kh, :, :, rl * W : (rl + 1) * W, :]
                    )
                    if kh == 0:
                        nc.sync.dma_start(
                            wt[96:97, :], bt[rl * W : (rl + 1) * W, :]
                        )
                    wts.append(wt)
                for qg in range(W // 16):
                    ps = pspool.tile([B, 512], F32)
                    for qq in range(16):
                        q = qg * 16 + qq
                        for kh in range(3):
                            kd = 97 if kh == 0 else 96
                            off = ((rl + kh) * W + q) * B
                            nc.tensor.matmul(
                                ps[0:B, qq * 32 : (qq + 1) * 32],
                                x3[0:kd, off : off + B],
                                wts[kh][0:kd, q * C_OUT : (q + 1) * C_OUT],
                                start=(kh == 0),
                                stop=(kh == 2),
                            )
                    nc.vector.tensor_copy(
                        out_v[:, rl, qg * 16 : (qg + 1) * 16, :],
                        ps[0:B, :].rearrange("p (q o) -> p q o", q=16),
                    )
            nc.sync.dma_start(out[:, :, :, :], out_sb[0:B, :])
    nc.compile()
    return nc


def _shard(inputs):
    x = np.asarray(inputs["x"], np.float32)
    weight = np.asarray(inputs["weight"], np.float32)[0]
    bias = np.asarray(inputs["bias"], np.float32)[0]
    xp = np.pad(x, ((0, 0), (0, 0), (1, 1), (1, 1)))
    bias_t = np.ascontiguousarray(bias.reshape(C_OUT, H * W).T)
    in_maps = []
    for k in range(N_CORES):
        r0 = RP * k
        in_maps.append(
            {
                "xh": np.ascontiguousarray(
                    xp[:, :, r0 : r0 + RP + 2, :].transpose(1, 2, 3, 0)
                ),
                "w": np.ascontiguousarray(weight[:, LP * k : LP * (k + 1), :]),
                "bt": np.ascontiguousarray(bias_t[LP * k : LP * (k + 1), :]),
            }
        )
    return in_maps


def _get_nc():
    global _nc
    if _nc is None:
        _nc = _build()
    return _nc


def kernel(**inputs):
    nc = _get_nc()
    res = run_bass_kernel_spmd(nc, _shard(inputs), list(range(N_CORES)))
    return np.concatenate(
        [res.results[k]["out"] for k in range(N_CORES)], axis=2
    )
